# revision 44
# baseline (speedup 1.0000x reference)
"""fp8-hybrid variant: k-dims 0..255 via one fp8e4m3 DoubleRow matmul per
group (K=256 per instruction, 0.5 cycles/row), k-dims 256..1023 in fp16.

Measured max-rel-err of this split (numpy, exact same host quantization
the kernel uses): 1.741e-2 against the 2e-2 gate. Quantization happens
on the HOST, the PE multiplies the supplied fp8 values exactly
(e4m3 x e4m3 products are exact in fp32) and accumulates in fp32, so
the hardware error equals the numpy-measured value up to summation
order (~1e-7) and is deterministic run to run.

Everything else (single load ring, dummy warm-up bridge, two-phase
schedule, half-chain tail) as in kernel.py.
"""

import numpy as np
import ml_dtypes

import concourse.bass as bass
import concourse.mybir as mybir
from concourse.bass_utils import run_bass_kernel_spmd

B, IN_F, OUT_F = 8192, 1024, 1024
N_CORES = 8
M = B // N_CORES
P = 128
MB = 512
HB = MB // 2
KT = IN_F // P        # 8 k tiles total
K8 = 2                # first two k-tiles go fp8 DoubleRow
KF = KT - K8          # 6 fp16 k tiles (k = 2..7)
NT = OUT_F // P
NGROUPS = 16
WXC = OUT_F + MB      # packed fp16 slice: 512 x-mb0 cols + 1024 W cols

F16 = mybir.dt.float16
F32 = mybir.dt.float32
F8 = mybir.dt.float8e4
DR = mybir.MatmulPerfMode.DoubleRow


def build_program() -> bass.Bass:
    nc = bass.Bass()
    # fp16 slices for k-tiles 2..7: rows (k-2)*128..: [x-mb0 | W]
    wxa = nc.declare_dram_parameter("wxa", [KF * P, WXC], F16, isOutput=False)
    xb = nc.declare_dram_parameter("xb", [KF * P, MB], F16, isOutput=False)
    # fp8 payload for k-tiles 0,1: plane o = k-tile o (contraction
    # index k = o*128 + p)
    w8d = nc.declare_dram_parameter("w8", [P, K8, OUT_F], F8, isOutput=False)
    x8ad = nc.declare_dram_parameter("x8a", [P, K8, MB], F8, isOutput=False)
    x8bd = nc.declare_dram_parameter("x8b", [P, K8, MB], F8, isOutput=False)
    h1fd = nc.declare_dram_parameter("h1f", [P, K8, HB + P], F16, isOutput=False)
    bias = nc.declare_dram_parameter("bias", [P, NT], F32, isOutput=False)
    outT = nc.declare_dram_parameter("outT", [OUT_F, M], F16, isOutput=True)

    import contextlib

    with contextlib.ExitStack() as ctx:
        wxt_sb = [
            ctx.enter_context(nc.sbuf_tensor(f"wxt{k}", [P, WXC], F16))
            for k in range(KF)
        ]
        xb_sb = [
            ctx.enter_context(nc.sbuf_tensor(f"xbt{k}", [P, MB], F16))
            for k in range(KF)
        ]
        w8_sb = ctx.enter_context(nc.sbuf_tensor("w8t", [P, K8, OUT_F], F8))
        x8a_sb = ctx.enter_context(nc.sbuf_tensor("x8at", [P, K8, MB], F8))
        x8b_sb = ctx.enter_context(nc.sbuf_tensor("x8bt", [P, K8, MB], F8))
        ot_sb = [
            ctx.enter_context(nc.sbuf_tensor(f"ot{j}", [P, MB], F16))
            for j in range(8)
        ]
        h1f_sb = ctx.enter_context(nc.sbuf_tensor("h1ft", [P, K8, HB + P], F16))
        bias_sb = ctx.enter_context(nc.sbuf_tensor("bias_sb", [P, NT], F32))
        dummy_sb = ctx.enter_context(nc.sbuf_tensor("dummy_sb", [P, P], F16))
        ps = [
            ctx.enter_context(nc.psum_tensor(f"ps{b}", [P, MB], F32))
            for b in range(8)
        ]
        ld_b = ctx.enter_context(nc.semaphore("ld_b"))
        dm = ctx.enter_context(nc.semaphore("dm"))
        ld_8x = ctx.enter_context(nc.semaphore("ld_8x"))
        ld_8wa = ctx.enter_context(nc.semaphore("ld_8wa"))  # w8 nt0-3
        ld_8wb = ctx.enter_context(nc.semaphore("ld_8wb"))  # w8 nt4-7
        ld_8xb = ctx.enter_context(nc.semaphore("ld_8xb"))
        ld_h1 = ctx.enter_context(nc.semaphore("ld_h1"))
        ld_s = [ctx.enter_context(nc.semaphore(f"ld_s{k}")) for k in range(KF)]
        ld_xb = [ctx.enter_context(nc.semaphore(f"ld_xb{k}")) for k in range(KF)]
        mm = ctx.enter_context(nc.semaphore("mm"))
        mmh = ctx.enter_context(nc.semaphore("mmh"))
        ev = ctx.enter_context(nc.semaphore("ev"))
        ev_h = ctx.enter_context(nc.semaphore("ev_h"))
        st_sems = [ctx.enter_context(nc.semaphore(f"st{j}")) for j in range(8)]
        st_h = ctx.enter_context(nc.semaphore("st_h"))

        def store_ap(g):
            mb, nt = divmod(g, NT)
            return outT[nt * P:(nt + 1) * P, mb * MB:(mb + 1) * MB]

        with nc.Block(no_gpsimd_drain=True) as block:

            @block.sync
            def _(sync):
                # ALL loads on this one HWDGE ring, in first-use order.
                sync.dma_start(
                    out=x8a_sb[:, :, :], in_=x8ad[:, :, :],
                ).then_inc(ld_8x, 16)
                sync.dma_start(
                    out=w8_sb[:, :, 0:MB], in_=w8d[:, :, 0:MB],
                ).then_inc(ld_8wa, 16)
                sync.dma_start(
                    out=w8_sb[:, :, MB:OUT_F], in_=w8d[:, :, MB:OUT_F],
                ).then_inc(ld_8wb, 16)
                for k in range(KF):
                    sync.dma_start(
                        out=wxt_sb[k][:], in_=wxa[k * P:(k + 1) * P, :],
                    ).then_inc(ld_s[k], 16)
                sync.dma_start(
                    out=x8b_sb[:, :, :], in_=x8bd[:, :, :],
                ).then_inc(ld_8xb, 16)
                sync.dma_start(
                    out=h1f_sb[:, :, :], in_=h1fd[:, :, :],
                ).then_inc(ld_h1, 16)
                for k in range(KF):
                    sync.dma_start(
                        out=xb_sb[k][:], in_=xb[k * P:(k + 1) * P, :],
                    ).then_inc(ld_xb[k], 16)
                sync.wait_ge(ev_h, 1)
                sync.dma_start(
                    out=outT[7 * P:8 * P, MB:MB + HB],
                    in_=ot_sb[7][:, 0:HB],
                ).then_inc(st_h, 16)
                sync.wait_ge(st_h, 32)

            @block.scalar
            def _(scalar):
                for g in range(NGROUPS - 1):
                    scalar.wait_ge(ev, g + 1)
                    scalar.dma_start(
                        out=store_ap(g), in_=ot_sb[g % 8][:],
                    ).then_inc(st_sems[g % 8], 16)
                scalar.wait_ge(ev_h, 2)
                scalar.dma_start(
                    out=outT[7 * P:8 * P, MB + HB:2 * MB],
                    in_=ot_sb[7][:, HB:MB],
                ).then_inc(st_h, 16)
                for j in range(7):
                    scalar.wait_ge(st_sems[j], 32)
                scalar.wait_ge(st_sems[7], 16)
                scalar.wait_ge(st_h, 32)

            @block.gpsimd
            def _(gpsimd):
                gpsimd.dma_start(out=bias_sb[:], in_=bias[:]).then_inc(ld_b, 16)

            @block.tensor
            def _(tensor):
                # 35 dummies = ~3.75 us bridge: covers the median
                # first-tile gate (~3.3-3.5 us: issue + transfer + HBM
                # completion receipt) so the real stream starts with the
                # HAM activity window already satisfied and the PE at
                # 2.4 GHz. (A longer 37-dummy insurance bridge measured
                # ~0.7 us slower on clean draws without demonstrably
                # protecting noisy ones.)
                tensor.wait_ge(dm, 1)
                for _ in range(35):
                    tensor.matmul(
                        ps[0][:, 0:P], dummy_sb[:, 0:P], dummy_sb[:, 0:P],
                        start=True, stop=True,
                    )
                # Phase A (mb=0): one fp8 DoubleRow matmul covers
                # k=0..255 per bank, then k-outer fp16 sweeps.
                tensor.wait_ge(ld_8x, 16)
                tensor.wait_ge(ld_8wa, 16)
                for nt in range(NT):
                    if nt == 4:
                        tensor.wait_ge(ld_8wb, 16)
                    tensor.matmul(
                        ps[nt][:, :],
                        w8_sb[:, :, nt * P:(nt + 1) * P],
                        x8a_sb[:, :, :],
                        start=True, stop=False,
                        perf_mode=DR,
                    )
                for k in range(KF):
                    tensor.wait_ge(ld_s[k], 16)
                    for nt in range(NT):
                        inst = tensor.matmul(
                            ps[nt][:, :],
                            wxt_sb[k][:, MB + nt * P:MB + (nt + 1) * P],
                            wxt_sb[k][:, 0:MB],
                            start=False,
                            stop=(k == KF - 1),
                        )
                        if k == KF - 1:
                            inst.then_inc(mm, 1)
                # Phase B (mb=1): k-inner fp16 chains per group keep
                # completions staggered, but the DoubleRow start-matmuls
                # are issued in PAIRS one-to-two groups ahead of their
                # chains: each normal<->DR mode transition costs ~200 ns
                # of pipeline serialization, so pairing halves the
                # transition count without racing ahead of the phase-A
                # evictions (v11's fully-grouped sweep stalled on them).
                tensor.wait_ge(ld_8xb, 16)
                for k in range(KF):
                    tensor.wait_ge(ld_xb[k], 16)

                def b_dr(nt, lo, hi, bank=None):
                    tensor.matmul(
                        ps[bank if bank is not None else nt][:, 0:hi - lo],
                        w8_sb[:, :, nt * P:(nt + 1) * P],
                        x8b_sb[:, :, lo:hi],
                        start=True, stop=False,
                        perf_mode=DR,
                    )

                def b_chain(nt, lo, hi, bank=None, sem=mm):
                    inst = None
                    for k in range(KF):
                        inst = tensor.matmul(
                            ps[bank if bank is not None else nt][:, 0:hi - lo],
                            wxt_sb[k][:, MB + nt * P:MB + (nt + 1) * P],
                            xb_sb[k][:, lo:hi],
                            start=False,
                            stop=(k == KF - 1),
                        )
                    inst.then_inc(sem, 1)

                # DR pairs (gated on the bank's phase-A eviction),
                # interleaved with the fp16 chains.
                tensor.wait_ge(ev, 1)
                b_dr(0, 0, MB)
                tensor.wait_ge(ev, 2)
                b_dr(1, 0, MB)
                b_chain(0, 0, MB)
                tensor.wait_ge(ev, 3)
                b_dr(2, 0, MB)
                tensor.wait_ge(ev, 4)
                b_dr(3, 0, MB)
                b_chain(1, 0, MB)
                b_chain(2, 0, MB)
                tensor.wait_ge(ev, 5)
                b_dr(4, 0, MB)
                tensor.wait_ge(ev, 6)
                b_dr(5, 0, MB)
                b_chain(3, 0, MB)
                b_chain(4, 0, MB)
                tensor.wait_ge(ev, 7)
                b_dr(6, 0, MB)
                tensor.wait_ge(ev, 8)
                b_dr(7, 0, HB, bank=7)          # h0 half, bank 7
                b_chain(5, 0, MB)
                b_chain(6, 0, MB)
                b_chain(7, 0, HB, bank=7, sem=mmh)   # h0
                # h1 in bank 5: free once group 13 (bank 5) is
                # evicted. Its k0-1 contribution uses fp16 (small extra
                # payload) instead of a lone DoubleRow block - the h1
                # chain sits on the critical tail path and a single-MM
                # DR block costs ~400ns of mode-switch serialization.
                tensor.wait_ge(ev, NGROUPS - 2)
                tensor.wait_ge(ld_h1, 16)
                for ko in range(K8):
                    tensor.matmul(
                        ps[5][:, 0:HB],
                        h1f_sb[:, ko, HB:HB + P],
                        h1f_sb[:, ko, 0:HB],
                        start=(ko == 0), stop=False,
                    )
                b_chain(7, HB, MB, bank=5, sem=mmh)  # h1

            @block.vector
            def _(vector):
                vector.memset(dummy_sb[:], 0.0).then_inc(dm, 1)
                vector.wait_ge(ld_b, 16)
                for g in range(NGROUPS - 1):
                    mb, nt = divmod(g, NT)
                    vector.wait_ge(mm, g + 1)
                    if g >= 8:
                        vector.wait_ge(st_sems[g - 8], 16)
                    vector.tensor_scalar_add(
                        ot_sb[g % 8][:],
                        ps[g % 8][:, :],
                        bias_sb[:, nt:nt + 1],
                    ).then_inc(ev, 1)
                vector.wait_ge(st_sems[7], 16)
                for h in range(2):
                    vector.wait_ge(mmh, h + 1)
                    vector.tensor_scalar_add(
                        ot_sb[7][:, h * HB:(h + 1) * HB],
                        ps[7 if h == 0 else 5][:, 0:HB],
                        bias_sb[:, 7:8],
                    ).then_inc(ev_h, 1)

    return nc


_PROGRAM = None


def _get_program() -> bass.Bass:
    global _PROGRAM
    if _PROGRAM is None:
        _PROGRAM = build_program()
    return _PROGRAM


F8NP = ml_dtypes.float8_e4m3fn


def make_in_maps(x: np.ndarray, W: np.ndarray, b: np.ndarray) -> list[dict]:
    WT = W.T  # [IN_F, OUT_F] f32
    WT16 = WT[K8 * P:, :].astype(np.float16)
    # fp8 planes: plane o = k-tile o
    w8 = np.ascontiguousarray(
        np.stack([WT[o * P:(o + 1) * P, :] for o in range(K8)], axis=1)
        .astype(F8NP)
    )
    bias = np.ascontiguousarray(
        b.astype(np.float32, copy=False).reshape(NT, P).T
    )
    in_maps = []
    for c in range(N_CORES):
        xT = x[c * M:(c + 1) * M, :].T  # [IN_F, M] f32
        xT16 = xT[K8 * P:, :].astype(np.float16)
        wxa = np.ascontiguousarray(
            np.concatenate([xT16[:, 0:MB], WT16], axis=1)
        )
        xbm = np.ascontiguousarray(xT16[:, MB:M])
        x8a = np.ascontiguousarray(
            np.stack([xT[o * P:(o + 1) * P, 0:MB] for o in range(K8)], axis=1)
            .astype(F8NP)
        )
        x8b = np.ascontiguousarray(
            np.stack([xT[o * P:(o + 1) * P, MB:M] for o in range(K8)], axis=1)
            .astype(F8NP)
        )
        h1f = np.ascontiguousarray(
            np.stack(
                [np.concatenate(
                    [xT[o * P:(o + 1) * P, MB + HB:M],
                     WT[o * P:(o + 1) * P, 7 * P:8 * P]], axis=1)
                 for o in range(K8)], axis=1,
            ).astype(np.float16)
        )
        in_maps.append({
            "wxa": wxa, "xb": xbm, "w8": w8, "x8a": x8a, "x8b": x8b,
            "h1f": h1f, "bias": bias,
        })
    return in_maps


def assemble_output(results: list[dict]) -> np.ndarray:
    out = np.empty((B, OUT_F), dtype=np.float32)
    for c in range(N_CORES):
        out[c * M:(c + 1) * M, :] = results[c]["outT"].T.astype(np.float32)
    return out


def kernel(x: np.ndarray, W: np.ndarray, b: np.ndarray) -> np.ndarray:
    nc = _get_program()
    in_maps = make_in_maps(np.asarray(x), np.asarray(W), np.asarray(b))
    res = run_bass_kernel_spmd(nc, in_maps, list(range(N_CORES)))
    return assemble_output(res.results)
